# revision 55
# baseline (speedup 1.0000x reference)
"""CAM (channel attention) module kernel for Trainium2 (Bass/Tile).

Reference computation (per batch b):
    energy  = x_b @ x_b.T                      # [C, C], contraction over N
    att     = softmax(rowmax(energy) - energy) # row-wise over last axis
    out     = att @ x_b                        # [C, N]
    y_b     = gamma * out + x_b

Sharding: data-parallel over B across 8 NeuronCores (B=32 -> 4 per core),
gamma replicated, full CxC attention per core.

Identity used: softmax(rowmax(E) - E)[i,j] = exp(mn[i] - E[i,j]) / Z[i]
with mn[i] = min_j E[i,j], Z[i] = sum_j exp(mn[i] - E[i,j])  (shift
invariance of softmax; exact).

Pipeline (per ~60us batch period; 389.5us -> 258.4us measured on HW):
  - x streamed in by SWDGE cast-DMA straight to bf16 (f32 read from
    HBM, bf16 landed in SBUF), issued 2 iterations ahead on the
    otherwise-empty GpSimd queue so an issue blocked on a buffer can
    never starve another engine.
  - PE phases per batch are kept as dense same-mode bursts (fine
    interleaving of transpose-mode with matmuls measurably thrashes
    the weight-load pipeline): MM1 (upper-triangular bf16 + mirrored
    lower) -> next batch's transpose burst (fills the softmax window)
    -> tS transposes -> MM2 in fp8e4 DoubleRow (2 k-chunks/matmul,
    ~1.8x over bf16).
  - ACT: exp(mn-E), mirror staging, xt evacuations, fp8 casts of x at
    the iteration tail (in-order queues: a blocked op at the head
    stalls everything behind it).  DVE: E row-min, Z row-sums, 1/Z,
    tT evacuation, MM2 evacuation (*gamma/Z + residual).  GpSimd:
    load issues only.
  - Residual comes from the bf16 copy of x (rel err ~3e-3 << 2e-2 tol).

Layouts per batch (P=128 partitions):
    X16 [P, CO, N] bf16   c-natural (c = co*P + p)
    X8  [P, CO, N] fp8e4  MM2 moving operand
    xt  [P, 2, C]  bf16   n-on-partition pair chunks (PE transpose)
    E   [P, CO, C] f32    PSUM, i on partitions (4 banks)
    tS  [P, CO, C] bf16   exp(mn - E)
    tT  [P, CO, C] fp8e4  j on partitions
    ot  [P, 2, 512] f32   (tT.T @ X8)*rg + X16 -> y
"""

import numpy as np

P = 128

_CACHE = {}


def _build(Bs, C, N, **opts):
    import concourse.bass as bass  # noqa: F401
    import concourse.tile as tile
    import concourse.mybir as mybir
    from concourse import bacc
    from concourse.masks import make_identity

    F32 = mybir.dt.float32
    BF16 = mybir.dt.bfloat16
    FP8 = mybir.dt.float8e4
    AF = mybir.ActivationFunctionType
    ALU = mybir.AluOpType
    AX = mybir.AxisListType
    DR = mybir.MatmulPerfMode.DoubleRow

    assert C == 4 * P and N % 2048 == 0
    CO = C // P          # 4   i/j chunks of 128
    KC = N // P          # 32  n chunks of 128 (MM1 contraction)
    KH = KC // 2         # 16  transpose pair-groups
    NF = N // 512        # 8   n chunks of 512 (MM2 free dim)
    nc = bacc.Bacc(None, target_bir_lowering=False, debug=False)
    x_in = nc.dram_tensor("x", [Bs, C, N], F32, kind="ExternalInput")
    g_in = nc.dram_tensor("gamma", [1], F32, kind="ExternalInput")
    y_out = nc.dram_tensor("y", [Bs, C, N], F32, kind="ExternalOutput")

    with tile.TileContext(nc) as tc:
        with (
            tc.tile_pool(name="consts", bufs=1) as consts,
            tc.tile_pool(name="x16p", bufs=3) as x16p,
            tc.tile_pool(name="x8p", bufs=2) as x8p,
            tc.tile_pool(name="xtp", bufs=KH + 1) as xtp,
            tc.tile_pool(name="tsp", bufs=1) as tsp,
            tc.tile_pool(name="ttp", bufs=2) as ttp,
            tc.tile_pool(name="op", bufs=4) as op,
            tc.tile_pool(name="stats", bufs=2) as stats,
            tc.tile_pool(name="stgp", bufs=2) as stgp,
            tc.tile_pool(name="pse", bufs=1, space="PSUM") as pse,
            tc.tile_pool(name="psx", bufs=2, space="PSUM") as psx,
            tc.tile_pool(name="psa", bufs=2, space="PSUM") as psa,
        ):
            ident16 = consts.tile([P, P], BF16)
            make_identity(nc, ident16)
            ident32 = consts.tile([P, P], F32)
            make_identity(nc, ident32)
            g_sb = consts.tile([1, 1], F32)
            nc.sync.dma_start(g_sb[:, :], g_in[:].rearrange("(a b) -> a b", a=1))
            g_col = consts.tile([P, 1], F32)
            nc.gpsimd.partition_broadcast(g_col[:, :], g_sb[:1, :1])

            def emit_load(b, parts=2):
                """SWDGE cast-DMA x[b] f32 -> bf16 SBUF in n-slices.

                GpSimd's FIFO carries ONLY these issues, so nothing can
                deadlock or delay behind a buffer-blocked issue.
                """
                xv = x_in[b].rearrange("(co p) n -> p co n", p=P)
                X16 = x16p.tile([P, CO, N], BF16, tag="X16", name="X16")
                step = N // parts
                for h in range(parts):
                    s = slice(h * step, (h + 1) * step)
                    nc.gpsimd.dma_start(X16[:, :, s], xv[:, :, s])
                return X16

            def emit_x8(b, X16):
                """fp8 copy of x for MM2's moving operand (ACT)."""
                X8 = x8p.tile([P, CO, N], FP8, tag="X8", name="X8")
                for co in range(CO):
                    nc.scalar.copy(X8[:, co, :], X16[:, co, :])
                return X8

            def emit_T_group(b, X16, kh, evac="act"):
                """One pair-chunk (256 n-cols) of x transposed to xt.

                The psx evacuation engine is caller-chosen: the head
                groups (bridging the softmax window while ACT is busy
                with exp) evacuate on DVE; the groups spliced into MM2
                (while DVE drains stt) evacuate on ACT.
                """
                ps_x = psx.tile([P, 2, C], BF16, tag="psx", name="ps_x")
                for q in (0, 1):
                    kc = 2 * kh + q
                    ks = slice(kc * P, (kc + 1) * P)
                    for co in range(CO):
                        nc.tensor.transpose(
                            ps_x[:, q, co * P:(co + 1) * P],
                            X16[:, co, ks], ident16,
                        )
                xt = xtp.tile([P, 2, C], BF16, tag="xt", name="xt")
                if evac == "act":
                    nc.scalar.copy(xt[:, :, :], ps_x[:, :, :])
                else:
                    nc.vector.tensor_copy(xt[:, :, :], ps_x[:, :, :])
                return xt

            def emit_mm1(b, xts):
                """E = x x^T, upper-triangular blocks + mirrored lower."""
                E = pse.tile([P, CO, C], F32, tag="E", name="E")
                for kc in range(KC):
                    xk = xts[kc // 2][:, kc % 2, :]
                    for ic in range(CO):
                        nc.tensor.matmul(
                            E[:, ic, ic * P:],
                            xk[:, ic * P:(ic + 1) * P],
                            xk[:, ic * P:],
                            start=(kc == 0),
                            stop=(kc == KC - 1),
                        )
                for jc in range(1, CO):
                    for ic in range(jc):
                        st = stgp.tile([P, P], F32, tag="stg", name="st")
                        nc.scalar.copy(st[:, :], E[:, ic, jc * P:(jc + 1) * P])
                        nc.tensor.matmul(
                            E[:, jc, ic * P:(ic + 1) * P],
                            st[:, :],
                            ident32,
                            is_transpose=True,
                            skip_group_check=True,
                        )
                return E

            def emit_softmax(b, E):
                """tS = exp(mn - E) (bf16), Z row-sums fused, rg = gamma/Z."""
                mn = stats.tile([P, CO], F32, tag="mn", name="mn")
                zs = stats.tile([P, CO], F32, tag="zs", name="zs")
                rg = stats.tile([P, CO], F32, tag="rg", name="rg")
                tS = tsp.tile([P, CO, C], BF16, tag="tS", name="tS")
                for ic in range(CO):
                    nc.vector.tensor_reduce(
                        mn[:, ic:ic + 1], E[:, ic, :], AX.X, ALU.min
                    )
                for ic in range(CO):
                    # No accum_out: Z comes from a DVE reduce so the exp
                    # chain (which gates tT and hence MM2) stays short.
                    nc.scalar.activation(
                        tS[:, ic, :], E[:, ic, :], AF.Exp,
                        bias=mn[:, ic:ic + 1], scale=-1.0,
                    )
                for ic in range(CO):
                    nc.vector.tensor_reduce(
                        zs[:, ic:ic + 1], tS[:, ic, :], AX.X, ALU.add
                    )
                nc.vector.reciprocal(rg[:, :], zs[:, :])
                nc.vector.tensor_scalar_mul(rg[:, :], rg[:, :], g_col[:, :1])
                return tS, rg

            def emit_tT(b, tS):
                """tT[j, i] = tS[i, j], fp8 for DoubleRow MM2."""
                tT = ttp.tile([P, CO, C], FP8, tag="tT", name="tT")
                for jh in range(CO // 2):
                    ps_t = psa.tile([P, 2, C], BF16, tag="acc", name="ps_t")
                    for q in (0, 1):
                        jc = 2 * jh + q
                        js = slice(jc * P, (jc + 1) * P)
                        for ic in range(CO):
                            nc.tensor.transpose(
                                ps_t[:, q, ic * P:(ic + 1) * P],
                                tS[:, ic, js], ident16,
                            )
                    nc.vector.tensor_copy(tT[:, 2 * jh:2 * jh + 2, :],
                                          ps_t[:, :, :])
                return tT

            def emit_mm2(b, tT, X8, X16, rg, fillers=()):
                """out = att @ x (fp8 DoubleRow), *gamma/Z + residual, store.

                `fillers` are deferred emit-closures (next batch's
                transpose groups), spliced in blocks of 3 after every
                8th MM2 group: coarse enough to keep the weight-load
                pipeline streaming within each mode, fine enough that
                the PE never leaves HAM's activity window, with slack
                for the DVE evacuations to keep up.
                """
                fillers = list(fillers)
                y_b = y_out[b].rearrange("(co p) n -> p co n", p=P)
                group = 0
                out = []
                for ic in range(CO):
                    # last row-block of the last batch: per-512 output
                    # DMAs so the final store drains right behind the
                    # last evacuation instead of waiting on a 1MB tile
                    ww = 1 if (b == Bs - 1 and ic == CO - 1) else 2
                    for nh in range(NF // ww):
                        ot = op.tile([P, ww, 512], F32, tag="o", name="ot")
                        for nj in range(ww):
                            nf = nh * ww + nj
                            ns = slice(nf * 512, (nf + 1) * 512)
                            ps2 = psa.tile([P, 512], F32, tag="acc",
                                           name="ps2")
                            for q in (0, 1):
                                nc.tensor.matmul(
                                    ps2[:, :],
                                    tT[:, 2 * q:2 * q + 2,
                                       ic * P:(ic + 1) * P],
                                    X8[:, 2 * q:2 * q + 2, ns],
                                    start=(q == 0),
                                    stop=(q == 1),
                                    perf_mode=DR,
                                )
                            nc.vector.scalar_tensor_tensor(
                                ot[:, nj, :], ps2[:, :], rg[:, ic:ic + 1],
                                X16[:, ic, ns],
                                op0=ALU.mult, op1=ALU.add,
                            )
                            group += 1
                            if group % 8 == 0:
                                for _ in range(3):
                                    if fillers:
                                        out.append(fillers.pop(0)())
                        nc.sync.dma_start(
                            y_b[:, ic, nh * ww * 512:(nh + 1) * ww * 512],
                            ot[:, :, :],
                        )
                while fillers:
                    out.append(fillers.pop(0)())
                return out

            # ---- software pipeline over batches ----
            # Loads issued 2 iterations ahead on the otherwise-empty
            # GpSimd queue; X8 casts at each iteration's tail so they
            # never block evacuations in the in-order ACT queue.
            X16s = {0: emit_load(0, parts=8)}
            if Bs > 1:
                X16s[1] = emit_load(1)
            xts = {0: [emit_T_group(0, X16s[0], kh) for kh in range(KH)]}
            X8s = {0: emit_x8(0, X16s[0])}
            for b in range(Bs):
                if b + 2 < Bs:
                    # single full-batch transfer: best DMA efficiency;
                    # arrival slack is ~1 full period with 2-ahead issue
                    X16s[b + 2] = emit_load(b + 2, parts=1)
                E = emit_mm1(b, xts.pop(b))
                head = []
                if b + 1 < Bs:
                    # First 2 next-batch transpose groups land before the
                    # softmax in emission order, evacuating on DVE ahead
                    # of the min/zs chain — the burst isn't gated on the
                    # ACT exp chain, so the PE rolls from MM1 straight
                    # into it.
                    head = [emit_T_group(b + 1, X16s[b + 1], kh,
                                         evac="dve")
                            for kh in range(4)]
                tS, rg = emit_softmax(b, E)
                if b + 1 < Bs:
                    xts[b + 1] = head + [
                        emit_T_group(b + 1, X16s[b + 1], kh)
                        for kh in range(4, KH)
                    ]
                tT = emit_tT(b, tS)
                emit_mm2(b, tT, X8s.pop(b), X16s[b], rg)
                del X16s[b]
                if b + 1 < Bs:
                    X8s[b + 1] = emit_x8(b + 1, X16s[b + 1])

    nc.compile()
    return nc


def get_nc(Bs=4, C=512, N=4096, **opts):
    key = (Bs, C, N, tuple(sorted(opts.items())))
    if key not in _CACHE:
        _CACHE[key] = _build(Bs, C, N, **opts)
    return _CACHE[key]


def kernel(x, gamma):
    """Full inputs in, full output out. x [32, 512, 4096] f32, gamma [1] f32."""
    from concourse.bass_utils import run_bass_kernel_spmd

    x = np.ascontiguousarray(np.asarray(x, dtype=np.float32))
    gamma = np.ascontiguousarray(np.asarray(gamma, dtype=np.float32))
    B, C, N = x.shape
    n_cores = 8
    assert B % n_cores == 0
    Bs = B // n_cores

    nc = get_nc(Bs, C, N)
    in_maps = [
        {"x": x[i * Bs:(i + 1) * Bs], "gamma": gamma} for i in range(n_cores)
    ]
    res = run_bass_kernel_spmd(nc, in_maps, core_ids=list(range(n_cores)))
    return np.concatenate([r["y"] for r in res.results], axis=0)


# revision 57
# speedup vs baseline: 1.0173x; 1.0173x over previous
"""CAM (channel attention) module kernel for Trainium2 (Bass/Tile).

Reference computation (per batch b):
    energy  = x_b @ x_b.T                      # [C, C], contraction over N
    att     = softmax(rowmax(energy) - energy) # row-wise over last axis
    out     = att @ x_b                        # [C, N]
    y_b     = gamma * out + x_b

Sharding: data-parallel over B across 8 NeuronCores (B=32 -> 4 per core),
gamma replicated, full CxC attention per core.

Identity used: softmax(rowmax(E) - E)[i,j] = exp(mn[i] - E[i,j]) / Z[i]
with mn[i] = min_j E[i,j], Z[i] = sum_j exp(mn[i] - E[i,j])  (shift
invariance of softmax; exact).

Pipeline (per ~60us batch period; 389.5us -> 258.4us measured on HW):
  - x streamed in by SWDGE cast-DMA straight to bf16 (f32 read from
    HBM, bf16 landed in SBUF), issued 2 iterations ahead on the
    otherwise-empty GpSimd queue so an issue blocked on a buffer can
    never starve another engine.
  - PE phases per batch are kept as dense same-mode bursts (fine
    interleaving of transpose-mode with matmuls measurably thrashes
    the weight-load pipeline): MM1 (upper-triangular bf16 + mirrored
    lower) -> next batch's transpose burst (fills the softmax window)
    -> tS transposes -> MM2 in fp8e4 DoubleRow (2 k-chunks/matmul,
    ~1.8x over bf16).
  - ACT: exp(mn-E), mirror staging, xt evacuations, fp8 casts of x at
    the iteration tail (in-order queues: a blocked op at the head
    stalls everything behind it).  DVE: E row-min, Z row-sums, 1/Z,
    tT evacuation, MM2 evacuation (*gamma/Z + residual).  GpSimd:
    load issues only.
  - Residual comes from the bf16 copy of x (rel err ~3e-3 << 2e-2 tol).

Layouts per batch (P=128 partitions):
    X16 [P, CO, N] bf16   c-natural (c = co*P + p)
    X8  [P, CO, N] fp8e4  MM2 moving operand
    xt  [P, 2, C]  bf16   n-on-partition pair chunks (PE transpose)
    E   [P, CO, C] f32    PSUM, i on partitions (4 banks)
    tS  [P, CO, C] bf16   exp(mn - E)
    tT  [P, CO, C] fp8e4  j on partitions
    ot  [P, 2, 512] f32   (tT.T @ X8)*rg + X16 -> y
"""

import numpy as np

P = 128

_CACHE = {}


def _build(Bs, C, N, **opts):
    import concourse.bass as bass  # noqa: F401
    import concourse.tile as tile
    import concourse.mybir as mybir
    from concourse import bacc
    from concourse.masks import make_identity

    F32 = mybir.dt.float32
    BF16 = mybir.dt.bfloat16
    FP8 = mybir.dt.float8e4
    AF = mybir.ActivationFunctionType
    ALU = mybir.AluOpType
    AX = mybir.AxisListType
    DR = mybir.MatmulPerfMode.DoubleRow

    assert C == 4 * P and N % 2048 == 0
    CO = C // P          # 4   i/j chunks of 128
    KC = N // P          # 32  n chunks of 128 (MM1 contraction)
    KH = KC // 2         # 16  transpose pair-groups
    NF = N // 512        # 8   n chunks of 512 (MM2 free dim)
    nc = bacc.Bacc(None, target_bir_lowering=False, debug=False)
    x_in = nc.dram_tensor("x", [Bs, C, N], F32, kind="ExternalInput")
    g_in = nc.dram_tensor("gamma", [1], F32, kind="ExternalInput")
    y_out = nc.dram_tensor("y", [Bs, C, N], F32, kind="ExternalOutput")

    with tile.TileContext(nc) as tc:
        with (
            tc.tile_pool(name="consts", bufs=1) as consts,
            tc.tile_pool(name="x16p", bufs=3) as x16p,
            tc.tile_pool(name="x8p", bufs=2) as x8p,
            tc.tile_pool(name="xtp", bufs=KH + 1) as xtp,
            tc.tile_pool(name="tsp", bufs=1) as tsp,
            tc.tile_pool(name="ttp", bufs=2) as ttp,
            tc.tile_pool(name="op", bufs=4) as op,
            tc.tile_pool(name="stats", bufs=2) as stats,
            tc.tile_pool(name="stgp", bufs=2) as stgp,
            tc.tile_pool(name="pse", bufs=1, space="PSUM") as pse,
            tc.tile_pool(name="psx", bufs=2, space="PSUM") as psx,
            tc.tile_pool(name="psa", bufs=2, space="PSUM") as psa,
        ):
            ident16 = consts.tile([P, P], BF16)
            make_identity(nc, ident16)
            ident32 = consts.tile([P, P], F32)
            make_identity(nc, ident32)
            g_sb = consts.tile([1, 1], F32)
            nc.sync.dma_start(g_sb[:, :], g_in[:].rearrange("(a b) -> a b", a=1))
            g_col = consts.tile([P, 1], F32)
            nc.gpsimd.partition_broadcast(g_col[:, :], g_sb[:1, :1])

            def emit_load(b, parts=2):
                """SWDGE cast-DMA x[b] f32 -> bf16 SBUF in n-slices.

                GpSimd's FIFO carries ONLY these issues, so nothing can
                deadlock or delay behind a buffer-blocked issue.
                """
                xv = x_in[b].rearrange("(co p) n -> p co n", p=P)
                X16 = x16p.tile([P, CO, N], BF16, tag="X16", name="X16")
                step = N // parts
                for h in range(parts):
                    s = slice(h * step, (h + 1) * step)
                    nc.gpsimd.dma_start(X16[:, :, s], xv[:, :, s])
                return X16

            def emit_x8(b, X16):
                """fp8 copy of x for MM2's moving operand (ACT)."""
                X8 = x8p.tile([P, CO, N], FP8, tag="X8", name="X8")
                for co in range(CO):
                    nc.scalar.copy(X8[:, co, :], X16[:, co, :])
                return X8

            def emit_T_group(b, X16, kh, evac="act"):
                """One pair-chunk (256 n-cols) of x transposed to xt.

                The psx evacuation engine is caller-chosen: the head
                groups (bridging the softmax window while ACT is busy
                with exp) evacuate on DVE; the groups spliced into MM2
                (while DVE drains stt) evacuate on ACT.
                """
                ps_x = psx.tile([P, 2, C], BF16, tag="psx", name="ps_x")
                for q in (0, 1):
                    kc = 2 * kh + q
                    ks = slice(kc * P, (kc + 1) * P)
                    for co in range(CO):
                        nc.tensor.transpose(
                            ps_x[:, q, co * P:(co + 1) * P],
                            X16[:, co, ks], ident16,
                        )
                xt = xtp.tile([P, 2, C], BF16, tag="xt", name="xt")
                if evac == "act":
                    nc.scalar.copy(xt[:, :, :], ps_x[:, :, :])
                else:
                    nc.vector.tensor_copy(xt[:, :, :], ps_x[:, :, :])
                return xt

            def emit_mm1(b, xts):
                """E = x x^T, upper-triangular blocks + mirrored lower."""
                E = pse.tile([P, CO, C], F32, tag="E", name="E")
                for kc in range(KC):
                    xk = xts[kc // 2][:, kc % 2, :]
                    for ic in range(CO):
                        nc.tensor.matmul(
                            E[:, ic, ic * P:],
                            xk[:, ic * P:(ic + 1) * P],
                            xk[:, ic * P:],
                            start=(kc == 0),
                            stop=(kc == KC - 1),
                        )
                for jc in range(1, CO):
                    for ic in range(jc):
                        st = stgp.tile([P, P], F32, tag="stg", name="st")
                        nc.scalar.copy(st[:, :], E[:, ic, jc * P:(jc + 1) * P])
                        nc.tensor.matmul(
                            E[:, jc, ic * P:(ic + 1) * P],
                            st[:, :],
                            ident32,
                            is_transpose=True,
                            skip_group_check=True,
                        )
                return E

            def emit_softmax(b, E):
                """tS = exp(mn - E) (bf16), Z row-sums fused, rg = gamma/Z."""
                mn = stats.tile([P, CO], F32, tag="mn", name="mn")
                zs = stats.tile([P, CO], F32, tag="zs", name="zs")
                rg = stats.tile([P, CO], F32, tag="rg", name="rg")
                tS = tsp.tile([P, CO, C], BF16, tag="tS", name="tS")
                for ic in range(CO):
                    nc.vector.tensor_reduce(
                        mn[:, ic:ic + 1], E[:, ic, :], AX.X, ALU.min
                    )
                for ic in range(CO):
                    # No accum_out: Z comes from a DVE reduce so the exp
                    # chain (which gates tT and hence MM2) stays short.
                    nc.scalar.activation(
                        tS[:, ic, :], E[:, ic, :], AF.Exp,
                        bias=mn[:, ic:ic + 1], scale=-1.0,
                    )
                for ic in range(CO):
                    nc.vector.tensor_reduce(
                        zs[:, ic:ic + 1], tS[:, ic, :], AX.X, ALU.add
                    )
                nc.vector.reciprocal(rg[:, :], zs[:, :])
                nc.vector.tensor_scalar_mul(rg[:, :], rg[:, :], g_col[:, :1])
                return tS, rg

            def emit_tT(b, tS):
                """tT[j, i] = tS[i, j], fp8 for DoubleRow MM2."""
                tT = ttp.tile([P, CO, C], FP8, tag="tT", name="tT")
                for jh in range(CO // 2):
                    ps_t = psa.tile([P, 2, C], BF16, tag="acc", name="ps_t")
                    for q in (0, 1):
                        jc = 2 * jh + q
                        js = slice(jc * P, (jc + 1) * P)
                        for ic in range(CO):
                            nc.tensor.transpose(
                                ps_t[:, q, ic * P:(ic + 1) * P],
                                tS[:, ic, js], ident16,
                            )
                    nc.vector.tensor_copy(tT[:, 2 * jh:2 * jh + 2, :],
                                          ps_t[:, :, :])
                return tT

            def emit_mm2(b, tT, X8, X16, rg, fillers=()):
                """out = att @ x (fp8 DoubleRow), *gamma/Z + residual, store.

                `fillers` are deferred emit-closures (next batch's
                transpose groups), spliced in blocks of 3 after every
                8th MM2 group: coarse enough to keep the weight-load
                pipeline streaming within each mode, fine enough that
                the PE never leaves HAM's activity window, with slack
                for the DVE evacuations to keep up.
                """
                fillers = list(fillers)
                y_b = y_out[b].rearrange("(co p) n -> p co n", p=P)
                group = 0
                out = []
                for ic in range(CO):
                    # last row-block of the last batch: per-512 output
                    # DMAs so the final store drains right behind the
                    # last evacuation instead of waiting on a 1MB tile
                    ww = 1 if (b == Bs - 1 and ic == CO - 1) else 2
                    for nh in range(NF // ww):
                        ot = op.tile([P, ww, 512], F32, tag="o", name="ot")
                        for nj in range(ww):
                            nf = nh * ww + nj
                            ns = slice(nf * 512, (nf + 1) * 512)
                            ps2 = psa.tile([P, 512], F32, tag="acc",
                                           name="ps2")
                            for q in (0, 1):
                                nc.tensor.matmul(
                                    ps2[:, :],
                                    tT[:, 2 * q:2 * q + 2,
                                       ic * P:(ic + 1) * P],
                                    X8[:, 2 * q:2 * q + 2, ns],
                                    start=(q == 0),
                                    stop=(q == 1),
                                    perf_mode=DR,
                                )
                            nc.vector.scalar_tensor_tensor(
                                ot[:, nj, :], ps2[:, :], rg[:, ic:ic + 1],
                                X16[:, ic, ns],
                                op0=ALU.mult, op1=ALU.add,
                            )
                            group += 1
                            if group % 8 == 0:
                                for _ in range(3):
                                    if fillers:
                                        out.append(fillers.pop(0)())
                        nc.sync.dma_start(
                            y_b[:, ic, nh * ww * 512:(nh + 1) * ww * 512],
                            ot[:, :, :],
                        )
                while fillers:
                    out.append(fillers.pop(0)())
                return out

            # ---- software pipeline over batches ----
            # Loads issued 2 iterations ahead on the otherwise-empty
            # GpSimd queue; X8 casts at each iteration's tail so they
            # never block evacuations in the in-order ACT queue.
            X16s = {0: emit_load(0, parts=8)}
            if Bs > 1:
                X16s[1] = emit_load(1)
            xts = {0: [emit_T_group(0, X16s[0], kh) for kh in range(KH)]}
            X8s = {0: emit_x8(0, X16s[0])}
            for b in range(Bs):
                if b + 2 < Bs:
                    X16s[b + 2] = emit_load(b + 2)
                E = emit_mm1(b, xts.pop(b))
                head = []
                if b + 1 < Bs:
                    # First 2 next-batch transpose groups land before the
                    # softmax in emission order, evacuating on DVE ahead
                    # of the min/zs chain — the burst isn't gated on the
                    # ACT exp chain, so the PE rolls from MM1 straight
                    # into it.
                    head = [emit_T_group(b + 1, X16s[b + 1], kh,
                                         evac="dve")
                            for kh in range(6)]
                tS, rg = emit_softmax(b, E)
                if b + 1 < Bs:
                    xts[b + 1] = head + [
                        emit_T_group(b + 1, X16s[b + 1], kh)
                        for kh in range(6, KH)
                    ]
                tT = emit_tT(b, tS)
                emit_mm2(b, tT, X8s.pop(b), X16s[b], rg)
                del X16s[b]
                if b + 1 < Bs:
                    X8s[b + 1] = emit_x8(b + 1, X16s[b + 1])

    nc.compile()
    return nc


def get_nc(Bs=4, C=512, N=4096, **opts):
    key = (Bs, C, N, tuple(sorted(opts.items())))
    if key not in _CACHE:
        _CACHE[key] = _build(Bs, C, N, **opts)
    return _CACHE[key]


def kernel(x, gamma):
    """Full inputs in, full output out. x [32, 512, 4096] f32, gamma [1] f32."""
    from concourse.bass_utils import run_bass_kernel_spmd

    x = np.ascontiguousarray(np.asarray(x, dtype=np.float32))
    gamma = np.ascontiguousarray(np.asarray(gamma, dtype=np.float32))
    B, C, N = x.shape
    n_cores = 8
    assert B % n_cores == 0
    Bs = B // n_cores

    nc = get_nc(Bs, C, N)
    in_maps = [
        {"x": x[i * Bs:(i + 1) * Bs], "gamma": gamma} for i in range(n_cores)
    ]
    res = run_bass_kernel_spmd(nc, in_maps, core_ids=list(range(n_cores)))
    return np.concatenate([r["y"] for r in res.results], axis=0)


# revision 58
# speedup vs baseline: 1.0955x; 1.0769x over previous
"""CAM (channel attention) module kernel for Trainium2 (Bass/Tile).

Reference computation (per batch b):
    energy  = x_b @ x_b.T                      # [C, C], contraction over N
    att     = softmax(rowmax(energy) - energy) # row-wise over last axis
    out     = att @ x_b                        # [C, N]
    y_b     = gamma * out + x_b

Sharding: data-parallel over B across 8 NeuronCores (B=32 -> 4 per core),
gamma replicated, full CxC attention per core.

Identity used: softmax(rowmax(E) - E)[i,j] = exp(mn[i] - E[i,j]) / Z[i]
with mn[i] = min_j E[i,j], Z[i] = sum_j exp(mn[i] - E[i,j])  (shift
invariance of softmax; exact).

Pipeline (per ~60us batch period; 389.5us -> 258.4us measured on HW):
  - x streamed in by SWDGE cast-DMA straight to bf16 (f32 read from
    HBM, bf16 landed in SBUF), issued 2 iterations ahead on the
    otherwise-empty GpSimd queue so an issue blocked on a buffer can
    never starve another engine.
  - PE phases per batch are kept as dense same-mode bursts (fine
    interleaving of transpose-mode with matmuls measurably thrashes
    the weight-load pipeline): MM1 (upper-triangular bf16 + mirrored
    lower) -> next batch's transpose burst (fills the softmax window)
    -> tS transposes -> MM2 in fp8e4 DoubleRow (2 k-chunks/matmul,
    ~1.8x over bf16).
  - ACT: exp(mn-E), mirror staging, xt evacuations, fp8 casts of x at
    the iteration tail (in-order queues: a blocked op at the head
    stalls everything behind it).  DVE: E row-min, Z row-sums, 1/Z,
    tT evacuation, MM2 evacuation (*gamma/Z + residual).  GpSimd:
    load issues only.
  - Residual comes from the bf16 copy of x (rel err ~3e-3 << 2e-2 tol).

Layouts per batch (P=128 partitions):
    X16 [P, CO, N] bf16   c-natural (c = co*P + p)
    X8  [P, CO, N] fp8e4  MM2 moving operand
    xt  [P, 2, C]  bf16   n-on-partition pair chunks (PE transpose)
    E   [P, CO, C] f32    PSUM, i on partitions (4 banks)
    tS  [P, CO, C] bf16   exp(mn - E)
    tT  [P, CO, C] fp8e4  j on partitions
    ot  [P, 2, 512] f32   (tT.T @ X8)*rg + X16 -> y
"""

import numpy as np

P = 128

_CACHE = {}


def _build(Bs, C, N, **opts):
    import concourse.bass as bass  # noqa: F401
    import concourse.tile as tile
    import concourse.mybir as mybir
    from concourse import bacc
    from concourse.masks import make_identity

    F32 = mybir.dt.float32
    BF16 = mybir.dt.bfloat16
    FP8 = mybir.dt.float8e4
    AF = mybir.ActivationFunctionType
    ALU = mybir.AluOpType
    AX = mybir.AxisListType
    DR = mybir.MatmulPerfMode.DoubleRow

    assert C == 4 * P and N % 2048 == 0
    CO = C // P          # 4   i/j chunks of 128
    KC = N // P          # 32  n chunks of 128 (MM1 contraction)
    KH = KC // 2         # 16  transpose pair-groups
    NF = N // 512        # 8   n chunks of 512 (MM2 free dim)
    nc = bacc.Bacc(None, target_bir_lowering=False, debug=False)
    x_in = nc.dram_tensor("x", [Bs, C, N], F32, kind="ExternalInput")
    g_in = nc.dram_tensor("gamma", [1], F32, kind="ExternalInput")
    y_out = nc.dram_tensor("y", [Bs, C, N], F32, kind="ExternalOutput")

    with tile.TileContext(nc) as tc:
        with (
            tc.tile_pool(name="consts", bufs=1) as consts,
            tc.tile_pool(name="x16p", bufs=3) as x16p,
            tc.tile_pool(name="x8p", bufs=2) as x8p,
            tc.tile_pool(name="xtp", bufs=KH + 1) as xtp,
            tc.tile_pool(name="tsp", bufs=1) as tsp,
            tc.tile_pool(name="ttp", bufs=2) as ttp,
            tc.tile_pool(name="op", bufs=4) as op,
            tc.tile_pool(name="stats", bufs=2) as stats,
            tc.tile_pool(name="stgp", bufs=2) as stgp,
            tc.tile_pool(name="pse", bufs=1, space="PSUM") as pse,
            tc.tile_pool(name="psx", bufs=2, space="PSUM") as psx,
            tc.tile_pool(name="psa", bufs=2, space="PSUM") as psa,
        ):
            ident16 = consts.tile([P, P], BF16)
            make_identity(nc, ident16)
            ident32 = consts.tile([P, P], F32)
            make_identity(nc, ident32)
            g_sb = consts.tile([1, 1], F32)
            nc.sync.dma_start(g_sb[:, :], g_in[:].rearrange("(a b) -> a b", a=1))
            g_col = consts.tile([P, 1], F32)
            nc.gpsimd.partition_broadcast(g_col[:, :], g_sb[:1, :1])

            def emit_load(b, parts=2):
                """SWDGE cast-DMA x[b] f32 -> bf16 SBUF in n-slices.

                GpSimd's FIFO carries ONLY these issues, so nothing can
                deadlock or delay behind a buffer-blocked issue.
                """
                xv = x_in[b].rearrange("(co p) n -> p co n", p=P)
                X16 = x16p.tile([P, CO, N], BF16, tag="X16", name="X16")
                step = N // parts
                for h in range(parts):
                    s = slice(h * step, (h + 1) * step)
                    nc.gpsimd.dma_start(X16[:, :, s], xv[:, :, s])
                return X16

            def emit_x8(b, X16):
                """fp8 copy of x for MM2's moving operand (ACT)."""
                X8 = x8p.tile([P, CO, N], FP8, tag="X8", name="X8")
                for co in range(CO):
                    nc.scalar.copy(X8[:, co, :], X16[:, co, :])
                return X8

            def emit_T_group(b, X16, kh, evac="act"):
                """One pair-chunk (256 n-cols) of x transposed to xt.

                The psx evacuation engine is caller-chosen: the head
                groups (bridging the softmax window while ACT is busy
                with exp) evacuate on DVE; the groups spliced into MM2
                (while DVE drains stt) evacuate on ACT.
                """
                ps_x = psx.tile([P, 2, C], BF16, tag="psx", name="ps_x")
                for q in (0, 1):
                    kc = 2 * kh + q
                    ks = slice(kc * P, (kc + 1) * P)
                    for co in range(CO):
                        nc.tensor.transpose(
                            ps_x[:, q, co * P:(co + 1) * P],
                            X16[:, co, ks], ident16,
                        )
                xt = xtp.tile([P, 2, C], BF16, tag="xt", name="xt")
                if evac == "act":
                    nc.scalar.copy(xt[:, :, :], ps_x[:, :, :])
                else:
                    nc.vector.tensor_copy(xt[:, :, :], ps_x[:, :, :])
                return xt

            def emit_mm1(b, xts):
                """E = x x^T, upper-triangular blocks + mirrored lower."""
                E = pse.tile([P, CO, C], F32, tag="E", name="E")
                for kc in range(KC):
                    xk = xts[kc // 2][:, kc % 2, :]
                    for ic in range(CO):
                        nc.tensor.matmul(
                            E[:, ic, ic * P:],
                            xk[:, ic * P:(ic + 1) * P],
                            xk[:, ic * P:],
                            start=(kc == 0),
                            stop=(kc == KC - 1),
                        )
                for jc in range(1, CO):
                    for ic in range(jc):
                        st = stgp.tile([P, P], F32, tag="stg", name="st")
                        nc.scalar.copy(st[:, :], E[:, ic, jc * P:(jc + 1) * P])
                        nc.tensor.matmul(
                            E[:, jc, ic * P:(ic + 1) * P],
                            st[:, :],
                            ident32,
                            is_transpose=True,
                            skip_group_check=True,
                        )
                return E

            def emit_softmax(b, E):
                """tS = exp(mn - E) (bf16), Z row-sums fused, rg = gamma/Z."""
                mn = stats.tile([P, CO], F32, tag="mn", name="mn")
                zs = stats.tile([P, CO], F32, tag="zs", name="zs")
                rg = stats.tile([P, CO], F32, tag="rg", name="rg")
                tS = tsp.tile([P, CO, C], BF16, tag="tS", name="tS")
                for ic in range(CO):
                    nc.vector.tensor_reduce(
                        mn[:, ic:ic + 1], E[:, ic, :], AX.X, ALU.min
                    )
                for ic in range(CO):
                    # No accum_out: Z comes from a DVE reduce so the exp
                    # chain (which gates tT and hence MM2) stays short.
                    nc.scalar.activation(
                        tS[:, ic, :], E[:, ic, :], AF.Exp,
                        bias=mn[:, ic:ic + 1], scale=-1.0,
                    )
                for ic in range(CO):
                    nc.vector.tensor_reduce(
                        zs[:, ic:ic + 1], tS[:, ic, :], AX.X, ALU.add
                    )
                nc.vector.reciprocal(rg[:, :], zs[:, :])
                nc.vector.tensor_scalar_mul(rg[:, :], rg[:, :], g_col[:, :1])
                return tS, rg

            def emit_tT(b, tS):
                """tT[j, i] = tS[i, j], fp8 for DoubleRow MM2."""
                tT = ttp.tile([P, CO, C], FP8, tag="tT", name="tT")
                for jh in range(CO // 2):
                    ps_t = psa.tile([P, 2, C], BF16, tag="acc", name="ps_t")
                    for q in (0, 1):
                        jc = 2 * jh + q
                        js = slice(jc * P, (jc + 1) * P)
                        for ic in range(CO):
                            nc.tensor.transpose(
                                ps_t[:, q, ic * P:(ic + 1) * P],
                                tS[:, ic, js], ident16,
                            )
                    nc.vector.tensor_copy(tT[:, 2 * jh:2 * jh + 2, :],
                                          ps_t[:, :, :])
                return tT

            def emit_mm2(b, tT, X8, X16, rg, fillers=()):
                """out = att @ x (fp8 DoubleRow), *gamma/Z + residual, store.

                `fillers` are deferred emit-closures (next batch's
                transpose groups), spliced in blocks of 3 after every
                8th MM2 group: coarse enough to keep the weight-load
                pipeline streaming within each mode, fine enough that
                the PE never leaves HAM's activity window, with slack
                for the DVE evacuations to keep up.
                """
                fillers = list(fillers)
                y_b = y_out[b].rearrange("(co p) n -> p co n", p=P)
                group = 0
                out = []
                for ic in range(CO):
                    # last row-block of the last batch: per-512 output
                    # DMAs so the final store drains right behind the
                    # last evacuation instead of waiting on a 1MB tile
                    ww = 1 if (b == Bs - 1 and ic == CO - 1) else 2
                    for nh in range(NF // ww):
                        ot = op.tile([P, ww, 512], F32, tag="o", name="ot")
                        for nj in range(ww):
                            nf = nh * ww + nj
                            ns = slice(nf * 512, (nf + 1) * 512)
                            ps2 = psa.tile([P, 512], F32, tag="acc",
                                           name="ps2")
                            for q in (0, 1):
                                nc.tensor.matmul(
                                    ps2[:, :],
                                    tT[:, 2 * q:2 * q + 2,
                                       ic * P:(ic + 1) * P],
                                    X8[:, 2 * q:2 * q + 2, ns],
                                    start=(q == 0),
                                    stop=(q == 1),
                                    perf_mode=DR,
                                )
                            nc.vector.scalar_tensor_tensor(
                                ot[:, nj, :], ps2[:, :], rg[:, ic:ic + 1],
                                X16[:, ic, ns],
                                op0=ALU.mult, op1=ALU.add,
                            )
                            group += 1
                            if group % 8 == 0:
                                for _ in range(3):
                                    if fillers:
                                        out.append(fillers.pop(0)())
                        nc.sync.dma_start(
                            y_b[:, ic, nh * ww * 512:(nh + 1) * ww * 512],
                            ot[:, :, :],
                        )
                while fillers:
                    out.append(fillers.pop(0)())
                return out

            # ---- software pipeline over batches ----
            # Loads issued 2 iterations ahead on the otherwise-empty
            # GpSimd queue; X8 casts at each iteration's tail so they
            # never block evacuations in the in-order ACT queue.
            X16s = {0: emit_load(0, parts=8)}
            if Bs > 1:
                X16s[1] = emit_load(1)
            xts = {0: [emit_T_group(0, X16s[0], kh) for kh in range(KH)]}
            X8s = {0: emit_x8(0, X16s[0])}
            for b in range(Bs):
                if b + 2 < Bs:
                    X16s[b + 2] = emit_load(b + 2)
                E = emit_mm1(b, xts.pop(b))
                head = []
                if b + 1 < Bs:
                    # First 2 next-batch transpose groups land before the
                    # softmax in emission order, evacuating on DVE ahead
                    # of the min/zs chain — the burst isn't gated on the
                    # ACT exp chain, so the PE rolls from MM1 straight
                    # into it.
                    head = [emit_T_group(b + 1, X16s[b + 1], kh,
                                         evac="dve")
                            for kh in range(4)]
                tS, rg = emit_softmax(b, E)
                if b + 1 < Bs:
                    xts[b + 1] = head + [
                        emit_T_group(b + 1, X16s[b + 1], kh)
                        for kh in range(4, KH)
                    ]
                tT = emit_tT(b, tS)
                emit_mm2(b, tT, X8s.pop(b), X16s[b], rg)
                del X16s[b]
                if b + 1 < Bs:
                    X8s[b + 1] = emit_x8(b + 1, X16s[b + 1])

    nc.compile()
    return nc


def get_nc(Bs=4, C=512, N=4096, **opts):
    key = (Bs, C, N, tuple(sorted(opts.items())))
    if key not in _CACHE:
        _CACHE[key] = _build(Bs, C, N, **opts)
    return _CACHE[key]


def kernel(x, gamma):
    """Full inputs in, full output out. x [32, 512, 4096] f32, gamma [1] f32."""
    from concourse.bass_utils import run_bass_kernel_spmd

    x = np.ascontiguousarray(np.asarray(x, dtype=np.float32))
    gamma = np.ascontiguousarray(np.asarray(gamma, dtype=np.float32))
    B, C, N = x.shape
    n_cores = 8
    assert B % n_cores == 0
    Bs = B // n_cores

    nc = get_nc(Bs, C, N)
    in_maps = [
        {"x": x[i * Bs:(i + 1) * Bs], "gamma": gamma} for i in range(n_cores)
    ]
    res = run_bass_kernel_spmd(nc, in_maps, core_ids=list(range(n_cores)))
    return np.concatenate([r["y"] for r in res.results], axis=0)
